# revision 8
# baseline (speedup 1.0000x reference)
"""ALiBi sliding-window GQA attention on 8 Trainium2 NeuronCores.

Sharding: batch (2) x sequence quarter (4) -> 8 cores, each computing a
disjoint [512, 1024] output chunk from a 528-token input slice (16-token
halo on the left for the sliding window). No collectives needed.

Per-core kernel (bf16 compute, f32 accumulate), v2 dataflow:
  1. Q/K projections in [token, feature] layout, staged raw to SBUF via ACT
     copies; RMSNorm stats batched per activation function (no LUT thrash).
  2. Normalized Q/K transposed to [feature, token] via PE transpose-mode
     (no XBAR DMA transposes - those serialize ~1.2us each on SP/ACT).
  3. 5 query blocks of 112 with 128-key windows, heads in 2 half-phases
     (even heads then odd heads so packed PE row-groups never share a
     PSUM bank):
       scores come out PRE-TRANSPOSED: S.T[key, head-slot, query] =
         K^T.T @ Q^T, so softmax probabilities feed the AV matmul with no
         per-head transpose at all.
       P.T = exp(S.T/8) * ebT (host table: ALiBi slopes + causal/window
         mask as multiplicative zeros, pre-transposed, slot-permuted)
       row sums via ones-matmul (also replicates them across 64
         partitions); normalization by reciprocal-multiply fused into the
         AV PSUM->SBUF eviction.
  4. Output projection uses OUT^T as the stationary operand so results land
     in [token, feature] layout for contiguous stores.
"""

import math

import numpy as np
import ml_dtypes

import concourse.bass as bass
import concourse.tile as tile
from concourse import bacc, mybir
from concourse.bass_utils import run_bass_kernel_spmd

BF16 = ml_dtypes.bfloat16

B, L, DIM = 2, 2048, 1024
N_HEADS, N_KV_HEADS, HEAD_DIM = 16, 4, 64
WINDOW = 16
EPS = 0.01

LQ = 512           # queries per core
HALO = WINDOW      # left halo
NB = 5             # query blocks per core
BQ = 112           # queries per block
BK = 128           # key window per block
LX = HALO + NB * BQ  # 576 = padded x slice width per core
KT_LT = 96         # K projection l-tile
P = 128

# head h -> scores slot: even heads -> slots 0..7, odd -> 8..15 (pair order)
SLOT = [h // 2 + 8 * (h % 2) for h in range(N_HEADS)]

_compiled = None


def _alibi_slopes(n_heads):
    closest = 2 ** math.floor(math.log2(n_heads))
    base = 2.0 ** (-(2.0 ** (-(math.log2(closest) - 3))))
    slopes = base ** np.arange(1, closest + 1, dtype=np.float64)
    if closest < n_heads:
        eb = 2.0 ** (-(2.0 ** (-(math.log2(2 * closest) - 3))))
        extra = eb ** np.arange(1, 2 * (n_heads - closest) + 1, 2, dtype=np.float64)
        slopes = np.concatenate([slopes, extra])
    return slopes[:n_heads]


def _exp_bias_t(edge: bool) -> np.ndarray:
    """[BK, N_HEADS, BQ] transposed multiplicative softmax bias, slot order.

    Query i (block-local) sits at window column jk in [i, i+16]; entry is
    exp(slope_h * (jk - 16 - i)) inside the band, 0 outside.  With
    edge=True (first block of the sequence) keys at global position < 0
    (jk < 16) are additionally masked.
    """
    slopes = _alibi_slopes(N_HEADS)
    i = np.arange(BQ)[:, None]
    jk = np.arange(BK)[None, :]
    rel = jk - WINDOW - i                      # [BQ, BK]
    valid = (rel <= 0) & (rel >= -WINDOW)
    if edge:
        valid = valid & (jk >= WINDOW)
    arg = np.where(valid[None], slopes[:, None, None] * rel[None], -np.inf)
    eb = np.exp(arg)                           # [H, BQ, BK]
    perm = np.empty(N_HEADS, np.int64)
    perm[SLOT] = np.arange(N_HEADS)            # slot s holds head perm[s]
    return np.ascontiguousarray(eb[perm].transpose(2, 0, 1)).astype(BF16)


def _build(apply_u: bool):
    """Build the SPMD Bass program. apply_u folds q_norm_w*k_norm_w into K^T."""
    nc = bacc.Bacc("TRN2", target_bir_lowering=False, debug=False)
    f32, bf16 = mybir.dt.float32, mybir.dt.bfloat16

    xt_e = nc.dram_tensor("xt", [P, 8, LX], bf16, kind="ExternalInput")
    wq_e = nc.dram_tensor("wqT", [P, 8, 1024], bf16, kind="ExternalInput")
    wk_e = nc.dram_tensor("wkT", [P, 8, 256], bf16, kind="ExternalInput")
    wv_e = nc.dram_tensor("wvT", [P, 8, 256], bf16, kind="ExternalInput")
    wo_e = nc.dram_tensor("woT", [P, 8, 1024], bf16, kind="ExternalInput")
    eb0_e = nc.dram_tensor("ebT0", [BK, N_HEADS, BQ], bf16, kind="ExternalInput")
    ebr_e = nc.dram_tensor("ebTr", [BK, N_HEADS, BQ], bf16, kind="ExternalInput")
    id_e = nc.dram_tensor("ident", [BQ, BQ], bf16, kind="ExternalInput")
    u_e = nc.dram_tensor("uvec", [P, 1], f32, kind="ExternalInput")
    out_e = nc.dram_tensor("out", [LQ, DIM], f32, kind="ExternalOutput")

    NT = LX // KT_LT + 2 * NB  # 6 K-tiles + 10 Q-chunks

    with tile.TileContext(nc) as tc:
        with (
            tc.tile_pool(name="w", bufs=1) as wp,
            tc.tile_pool(name="glob", bufs=1) as gp,
            tc.tile_pool(name="raw", bufs=NT) as rp,
            tc.tile_pool(name="stage", bufs=4) as sp,
            tc.tile_pool(name="small", bufs=NT) as mp,
            tc.tile_pool(name="att", bufs=2) as ap,
            tc.tile_pool(name="pp", bufs=4, space="PSUM") as pp,
            tc.tile_pool(name="psc", bufs=1, space="PSUM") as psc,
            tc.tile_pool(name="pot", bufs=1, space="PSUM") as pot,
        ):
            # ---- input loads (per k-tile for fine-grained deps) ----
            xt = wp.tile([P, 8, LX], bf16)
            wkT = wp.tile([P, 8, 256], bf16)
            wqT = wp.tile([P, 8, 1024], bf16)
            wvT = wp.tile([P, 8, 256], bf16)
            woT = wp.tile([P, 8, 1024], bf16)
            for kt in range(8):
                nc.sync.dma_start(xt[:, kt], xt_e.ap()[:, kt])
                nc.sync.dma_start(wkT[:, kt], wk_e.ap()[:, kt])
                nc.sync.dma_start(wqT[:, kt], wq_e.ap()[:, kt])
                nc.sync.dma_start(wvT[:, kt], wv_e.ap()[:, kt])
            ebT0 = wp.tile([BK, N_HEADS, BQ], bf16)
            ebTr = wp.tile([BK, N_HEADS, BQ], bf16)
            ident = wp.tile([BQ, BQ], bf16)
            ones64 = wp.tile([P, 64], bf16)
            nc.sync.dma_start(ebT0[:], eb0_e.ap())
            nc.sync.dma_start(ebTr[:], ebr_e.ap())
            nc.sync.dma_start(ident[:], id_e.ap())
            nc.vector.memset(ones64[:], 1.0)
            uvec = wp.tile([P, 1], f32)
            if apply_u:
                nc.sync.dma_start(uvec[:], u_e.ap())
            for kt in range(8):
                nc.sync.dma_start(woT[:, kt], wo_e.ap()[:, kt])

            QT = gp.tile([P, 8, LX], bf16)    # normalized Q transposed
            KT = gp.tile([P, 2, LX], bf16)    # normalized K transposed
            OT = gp.tile([P, 8, LX], bf16)    # attention out transposed

            # ---- phase 1: K/Q projections -> raw SBUF (ACT Copy only) ----
            # tiles: 6 K l-tiles of 96, then 10 Q (block, half) chunks of
            # [112, 512]
            raws = []
            for lt in range(LX // KT_LT):
                k_ps = pp.tile([P, 512], f32, tag="pp", name="k_ps")[:KT_LT, :256]
                for kt in range(8):
                    nc.tensor.matmul(
                        k_ps, xt[:, kt, lt * KT_LT:(lt + 1) * KT_LT], wkT[:, kt],
                        start=(kt == 0), stop=(kt == 7),
                    )
                raw = rp.tile([BQ, 512], bf16, tag="raw", name="raw")[:KT_LT, :256]
                nc.scalar.copy(raw[:], k_ps[:])
                raws.append((raw, KT_LT, N_KV_HEADS))
            for b_ in range(NB):
                qs = b_ * BQ
                for ch in range(2):
                    q_ps = pp.tile([P, 512], f32, tag="pp", name="q_ps")[:BQ]
                    for kt in range(8):
                        nc.tensor.matmul(
                            q_ps,
                            xt[:, kt, HALO + qs:HALO + qs + BQ],
                            wqT[:, kt, ch * 512:(ch + 1) * 512],
                            start=(kt == 0), stop=(kt == 7),
                        )
                    raw = rp.tile([BQ, 512], bf16, tag="raw", name="q_raw")
                    nc.scalar.copy(raw[:], q_ps[:])
                    raws.append((raw, BQ, 8))

            # ---- phase 2: RMSNorm stats, batched per ACT function ----
            sqs, sss, rstds = [], [], []
            for raw, lpart, n_h in raws:
                sq = sp.tile([BQ, 512], bf16, tag="sq", name="sq")[:lpart, :n_h * 64]
                nc.scalar.square(sq[:], raw[:])
                sqs.append(sq)
            for (raw, lpart, n_h), sq in zip(raws, sqs):
                ss = mp.tile([BQ, 8], f32, tag="ss", name="ss")[:lpart, :n_h]
                nc.vector.reduce_sum(
                    ss[:], sq[:].rearrange("l (h d) -> l h d", d=HEAD_DIM),
                    axis=mybir.AxisListType.X,
                )
                nc.vector.tensor_scalar_add(ss[:], ss[:], HEAD_DIM * EPS)
                sss.append(ss)
            srts = []
            for (raw, lpart, n_h), ss in zip(raws, sss):
                srt = mp.tile([BQ, 8], f32, tag="srt", name="srt")[:lpart, :n_h]
                nc.scalar.activation(
                    srt[:], ss[:], mybir.ActivationFunctionType.Sqrt,
                    scale=1.0 / HEAD_DIM,
                )
                srts.append(srt)
            hats = []
            for (raw, lpart, n_h), srt in zip(raws, srts):
                rstd = mp.tile([BQ, 8], f32, tag="rstd", name="rstd")[:lpart, :n_h]
                nc.vector.reciprocal(rstd[:], srt[:])
                hat = rp.tile([BQ, 512], bf16, tag="hat", name="hat")[:lpart, :n_h * 64]
                nc.vector.tensor_tensor(
                    hat[:].rearrange("l (h d) -> l h d", d=HEAD_DIM),
                    raw[:].rearrange("l (h d) -> l h d", d=HEAD_DIM),
                    rstd[:, :, None].to_broadcast((lpart, n_h, HEAD_DIM)),
                    mybir.AluOpType.mult,
                )
                hats.append(hat)

            # ---- phase 3: PE transposes -> KT / QT ----
            idx = 0
            for lt in range(LX // KT_LT):
                hat = hats[idx]; idx += 1
                for ot in range(2):
                    tp = pp.tile([P, BQ], bf16, tag="pp", name="tp")[:, :KT_LT]
                    nc.tensor.transpose(
                        tp[:], hat[:, ot * P:(ot + 1) * P], ident[:KT_LT, :KT_LT])
                    dst = KT[:, ot, lt * KT_LT:(lt + 1) * KT_LT]
                    if (lt + ot) % 2 == 0:
                        nc.vector.tensor_copy(dst, tp[:])
                    else:
                        nc.scalar.copy(dst, tp[:])
            for b_ in range(NB):
                qs = b_ * BQ
                for ch in range(2):
                    hat = hats[idx]; idx += 1
                    for ot in range(4):
                        tp = pp.tile([P, BQ], bf16, tag="pp", name="tpq")
                        nc.tensor.transpose(
                            tp[:], hat[:, ot * P:(ot + 1) * P], ident[:])
                        dst = QT[:, ch * 4 + ot, qs:qs + BQ]
                        if ot % 2 == 0:
                            nc.vector.tensor_copy(dst, tp[:])
                        else:
                            nc.scalar.copy(dst, tp[:])
            if apply_u:
                kts = gp.tile([P, 2, LX], bf16)
                for ot in range(2):
                    nc.scalar.activation(
                        kts[:, ot], KT[:, ot],
                        mybir.ActivationFunctionType.Copy, scale=uvec[:],
                    )
                KT = kts

            # ---- phase 4: attention + output projection per block ----
            for b_ in range(NB):
                qs = b_ * BQ
                ebT = ebT0 if b_ == 0 else ebTr
                # V for this block's key window, [key, feat] layout
                v_ps = pp.tile([P, 512], f32, tag="pp", name="v_ps")[:, :256]
                for kt in range(8):
                    nc.tensor.matmul(
                        v_ps, xt[:, kt, qs:qs + BK], wvT[:, kt],
                        start=(kt == 0), stop=(kt == 7),
                    )
                vb = ap.tile([P, 256], bf16, tag="vb")
                nc.scalar.copy(vb[:], v_ps[:])

                ot_ps = pot.tile([P, 8, P], f32, tag="otps")
                rcps = []
                for half in range(2):
                    # scores, pre-transposed: S.T[jk, slot, i]
                    sc = psc.tile([P, 8, P], f32, tag="sc")
                    for t in range(8):
                        h = 2 * t + half
                        g = h % N_KV_HEADS
                        nc.tensor.matmul(
                            sc[:, t, :BQ],
                            KT[(g % 2) * 64:(g % 2) * 64 + 64, g // 2, qs:qs + BK],
                            QT[(h % 2) * 64:(h % 2) * 64 + 64, h // 2, qs:qs + BQ],
                            start=True, stop=True,
                        )
                    e_t = ap.tile([P, 8, BQ], bf16, tag="et")
                    nc.scalar.activation(
                        e_t[:], sc[:, :, :BQ],
                        mybir.ActivationFunctionType.Exp, scale=0.125,
                    )
                    ptr = ap.tile([P, 8, BQ], bf16, tag="ptr")
                    nc.vector.tensor_tensor(
                        ptr[:], e_t[:], ebT[:, half * 8:half * 8 + 8, :],
                        mybir.AluOpType.mult,
                    )
                    # AV: out rows (h%2)*64..+64 of pair t
                    for t in range(8):
                        h = 2 * t + half
                        g = h % N_KV_HEADS
                        nc.tensor.matmul(
                            ot_ps[half * 64:half * 64 + 64, t, :BQ],
                            vb[:, g * 64:(g + 1) * 64],
                            ptr[:, t, :],
                            start=True, stop=True,
                        )
                    # denominators, replicated over 64 partitions by the
                    # ones-matmul; reciprocal lands in SBUF for the fused
                    # normalize-evict multiply
                    rcp = sp.tile([64, 8, BQ], f32, tag="rcp", name="rcp")
                    for c in range(2):
                        den = pp.tile([P, 512], f32, tag="pp", name="den")[:64, :4 * BQ]
                        nc.tensor.matmul(
                            den, ones64[:], ptr[:, 4 * c:4 * c + 4, :],
                            start=True, stop=True,
                        )
                        nc.vector.reciprocal_approx_fast(
                            rcp[:, 4 * c:4 * c + 4, :],
                            den[:].rearrange("p (s i) -> p s i", i=BQ),
                        )
                    rcps.append(rcp)
                for half in range(2):
                    nc.vector.tensor_tensor(
                        OT[half * 64:half * 64 + 64, :, qs:qs + BQ],
                        ot_ps[half * 64:half * 64 + 64, :, :BQ],
                        rcps[half][:],
                        mybir.AluOpType.mult,
                    )

                # output projection for this block
                nrows = BQ if b_ < NB - 1 else LQ - (NB - 1) * BQ
                for ch in range(2):
                    y_ps = pp.tile([P, 512], f32, tag="pp", name="y_ps")[:BQ]
                    for ot in range(8):
                        nc.tensor.matmul(
                            y_ps, OT[:, ot, qs:qs + BQ],
                            woT[:, ot, ch * 512:(ch + 1) * 512],
                            start=(ot == 0), stop=(ot == 7),
                        )
                    y_sb = sp.tile([BQ, 512], f32, tag="ysb", name="ysb")
                    nc.scalar.copy(y_sb[:], y_ps[:])
                    nc.sync.dma_start(
                        out_e.ap()[qs:qs + nrows, ch * 512:(ch + 1) * 512],
                        y_sb[:nrows],
                    )
    nc.compile()
    return nc


def _shard_inputs(x, wq, wk, wv, wo, q_norm_w, k_norm_w):
    u = (np.asarray(q_norm_w, np.float32) * np.asarray(k_norm_w, np.float32))
    apply_u = not np.allclose(u, 1.0)

    def ktile(wT):  # [DIM, O] -> [128, 8, O] bf16 (k-tiled)
        return np.ascontiguousarray(
            wT.astype(BF16).reshape(8, P, -1).transpose(1, 0, 2))

    wqT = ktile(np.asarray(wq, np.float32).T)
    wkT = ktile(np.asarray(wk, np.float32).T)
    wvT = ktile(np.asarray(wv, np.float32).T)
    woT = ktile(np.asarray(wo, np.float32).T)  # wo[e, o] -> [o, e], contraction o
    uvec = np.tile(u, 2).reshape(P, 1).astype(np.float32)
    ebTr = _exp_bias_t(edge=False)
    ident = np.eye(BQ, dtype=np.float32).astype(BF16)

    in_maps = []
    for c in range(8):
        b, j = c // 4, c % 4
        xh = np.zeros((LX, DIM), np.float32)
        lo = j * LQ - HALO
        s0, s1 = max(lo, 0), min(j * LQ + NB * BQ, L)
        xh[s0 - lo:s1 - lo] = x[b, s0:s1]
        xtc = np.ascontiguousarray(
            xh.T.astype(BF16).reshape(8, P, LX).transpose(1, 0, 2))
        ebT0 = _exp_bias_t(edge=(j == 0))
        in_maps.append({
            "xt": xtc, "wqT": wqT, "wkT": wkT, "wvT": wvT, "woT": woT,
            "ebT0": ebT0, "ebTr": ebTr, "ident": ident, "uvec": uvec,
        })
    return in_maps, apply_u


def _run(inputs, trace=False):
    global _compiled
    in_maps, apply_u = _shard_inputs(**inputs)
    if _compiled is None or _compiled[1] != apply_u:
        _compiled = (_build(apply_u), apply_u)
    nc = _compiled[0]
    res = run_bass_kernel_spmd(nc, in_maps, list(range(8)), trace=trace)
    full = np.empty((B, L, DIM), np.float32)
    for c in range(8):
        b, j = c // 4, c % 4
        full[b, j * LQ:(j + 1) * LQ] = res.results[c]["out"]
    return full, res


def kernel(x, wq, wk, wv, wo, q_norm_w, k_norm_w):
    full, _ = _run(dict(x=np.asarray(x), wq=np.asarray(wq), wk=np.asarray(wk),
                        wv=np.asarray(wv), wo=np.asarray(wo),
                        q_norm_w=np.asarray(q_norm_w),
                        k_norm_w=np.asarray(k_norm_w)))
    return full


# revision 9
# speedup vs baseline: 1.0353x; 1.0353x over previous
"""ALiBi sliding-window GQA attention on 8 Trainium2 NeuronCores.

Sharding: batch (2) x sequence quarter (4) -> 8 cores, each computing a
disjoint [512, 1024] output chunk from a 528-token input slice (16-token
halo on the left for the sliding window). No collectives needed.

Per-core kernel (bf16 compute, f32 accumulate), v2 dataflow:
  1. Q/K projections in [token, feature] layout, staged raw to SBUF via ACT
     copies; RMSNorm stats batched per activation function (no LUT thrash).
  2. Normalized Q/K transposed to [feature, token] via PE transpose-mode
     (no XBAR DMA transposes - those serialize ~1.2us each on SP/ACT).
  3. 5 query blocks of 112 with 128-key windows, heads in 2 half-phases
     (even heads then odd heads so packed PE row-groups never share a
     PSUM bank):
       scores come out PRE-TRANSPOSED: S.T[key, head-slot, query] =
         K^T.T @ Q^T, so softmax probabilities feed the AV matmul with no
         per-head transpose at all.
       P.T = exp(S.T/8) * ebT (host table: ALiBi slopes + causal/window
         mask as multiplicative zeros, pre-transposed, slot-permuted)
       row sums via ones-matmul (also replicates them across 64
         partitions); normalization by reciprocal-multiply fused into the
         AV PSUM->SBUF eviction.
  4. Output projection uses OUT^T as the stationary operand so results land
     in [token, feature] layout for contiguous stores.
"""

import math

import numpy as np
import ml_dtypes

import concourse.bass as bass
import concourse.tile as tile
from concourse import bacc, mybir
from concourse.bass_utils import run_bass_kernel_spmd

BF16 = ml_dtypes.bfloat16

B, L, DIM = 2, 2048, 1024
N_HEADS, N_KV_HEADS, HEAD_DIM = 16, 4, 64
WINDOW = 16
EPS = 0.01

LQ = 512           # queries per core
HALO = WINDOW      # left halo
NB = 5             # query blocks per core
BQ = 112           # queries per block
BK = 128           # key window per block
LX = HALO + NB * BQ  # 576 = padded x slice width per core
KT_LT = 96         # K projection l-tile
P = 128

# head h -> scores slot: even heads -> slots 0..7, odd -> 8..15 (pair order)
SLOT = [h // 2 + 8 * (h % 2) for h in range(N_HEADS)]

_compiled = None


def _alibi_slopes(n_heads):
    closest = 2 ** math.floor(math.log2(n_heads))
    base = 2.0 ** (-(2.0 ** (-(math.log2(closest) - 3))))
    slopes = base ** np.arange(1, closest + 1, dtype=np.float64)
    if closest < n_heads:
        eb = 2.0 ** (-(2.0 ** (-(math.log2(2 * closest) - 3))))
        extra = eb ** np.arange(1, 2 * (n_heads - closest) + 1, 2, dtype=np.float64)
        slopes = np.concatenate([slopes, extra])
    return slopes[:n_heads]


def _exp_bias_t(edge: bool) -> np.ndarray:
    """[BK, N_HEADS, BQ] transposed multiplicative softmax bias, slot order.

    Query i (block-local) sits at window column jk in [i, i+16]; entry is
    exp(slope_h * (jk - 16 - i)) inside the band, 0 outside.  With
    edge=True (first block of the sequence) keys at global position < 0
    (jk < 16) are additionally masked.
    """
    slopes = _alibi_slopes(N_HEADS)
    i = np.arange(BQ)[:, None]
    jk = np.arange(BK)[None, :]
    rel = jk - WINDOW - i                      # [BQ, BK]
    valid = (rel <= 0) & (rel >= -WINDOW)
    if edge:
        valid = valid & (jk >= WINDOW)
    arg = np.where(valid[None], slopes[:, None, None] * rel[None], -np.inf)
    eb = np.exp(arg)                           # [H, BQ, BK]
    perm = np.empty(N_HEADS, np.int64)
    perm[SLOT] = np.arange(N_HEADS)            # slot s holds head perm[s]
    return np.ascontiguousarray(eb[perm].transpose(2, 0, 1)).astype(BF16)


def _build(apply_u: bool):
    """Build the SPMD Bass program. apply_u folds q_norm_w*k_norm_w into K^T."""
    nc = bacc.Bacc("TRN2", target_bir_lowering=False, debug=False)
    f32, bf16 = mybir.dt.float32, mybir.dt.bfloat16

    xt_e = nc.dram_tensor("xt", [P, 8, LX], bf16, kind="ExternalInput")
    wq_e = nc.dram_tensor("wqT", [P, 8, 1024], bf16, kind="ExternalInput")
    wk_e = nc.dram_tensor("wkT", [P, 8, 256], bf16, kind="ExternalInput")
    wv_e = nc.dram_tensor("wvT", [P, 8, 256], bf16, kind="ExternalInput")
    wo_e = nc.dram_tensor("woT", [P, 8, 1024], bf16, kind="ExternalInput")
    eb0_e = nc.dram_tensor("ebT0", [BK, N_HEADS, BQ], bf16, kind="ExternalInput")
    ebr_e = nc.dram_tensor("ebTr", [BK, N_HEADS, BQ], bf16, kind="ExternalInput")
    id_e = nc.dram_tensor("ident", [BQ, BQ], bf16, kind="ExternalInput")
    u_e = nc.dram_tensor("uvec", [P, 1], f32, kind="ExternalInput")
    out_e = nc.dram_tensor("out", [LQ, DIM], f32, kind="ExternalOutput")

    NT = LX // KT_LT + 2 * NB  # 6 K-tiles + 10 Q-chunks

    with tile.TileContext(nc) as tc:
        with (
            tc.tile_pool(name="w", bufs=1) as wp,
            tc.tile_pool(name="glob", bufs=1) as gp,
            tc.tile_pool(name="raw", bufs=NT) as rp,
            tc.tile_pool(name="stage", bufs=4) as sp,
            tc.tile_pool(name="small", bufs=NT) as mp,
            tc.tile_pool(name="att", bufs=2) as ap,
            tc.tile_pool(name="vpool", bufs=NB) as vp,
            tc.tile_pool(name="pp", bufs=2, space="PSUM") as pp,
            tc.tile_pool(name="pbig", bufs=3, space="PSUM") as pbig,
        ):
            # ---- PE warmup: junk matmuls fill the DMA-bound prologue so
            # the HAM clock-gate opens before real work arrives ----
            junk = wp.tile([P, 512], bf16)
            nc.vector.memset(junk[:], 1.0)
            ones64 = wp.tile([P, 64], bf16)
            nc.vector.memset(ones64[:], 1.0)
            wps = pp.tile([P, 512], f32, tag="pp", name="warm")[:64]
            for _ in range(16):
                nc.tensor.matmul(wps, ones64[:], junk[:], start=True, stop=True)

            # ---- input loads (per k-tile for fine-grained deps) ----
            xt = wp.tile([P, 8, LX], bf16)
            wkT = wp.tile([P, 8, 256], bf16)
            wqT = wp.tile([P, 8, 1024], bf16)
            wvT = wp.tile([P, 8, 256], bf16)
            woT = wp.tile([P, 8, 1024], bf16)
            for kt in range(8):
                nc.sync.dma_start(xt[:, kt], xt_e.ap()[:, kt])
                nc.sync.dma_start(wkT[:, kt], wk_e.ap()[:, kt])
                nc.sync.dma_start(wvT[:, kt], wv_e.ap()[:, kt])
                nc.sync.dma_start(wqT[:, kt], wq_e.ap()[:, kt])
            ebT0 = wp.tile([BK, N_HEADS, BQ], bf16)
            ebTr = wp.tile([BK, N_HEADS, BQ], bf16)
            ident = wp.tile([BQ, BQ], bf16)
            nc.sync.dma_start(ebT0[:], eb0_e.ap())
            nc.sync.dma_start(ebTr[:], ebr_e.ap())
            nc.sync.dma_start(ident[:], id_e.ap())
            uvec = wp.tile([P, 1], f32)
            if apply_u:
                nc.sync.dma_start(uvec[:], u_e.ap())
            for kt in range(8):
                nc.sync.dma_start(woT[:, kt], wo_e.ap()[:, kt])

            QT = gp.tile([P, 8, LX], bf16)    # normalized Q transposed
            KT = gp.tile([P, 2, LX], bf16)    # normalized K transposed
            OT = gp.tile([P, 8, LX], bf16)    # attention out transposed

            # ---- phase 1: K/Q projections -> raw SBUF (ACT Copy only) ----
            # tiles: 6 K l-tiles of 96, then 10 Q (block, half) chunks of
            # [112, 512]
            raws = []
            for lt in range(LX // KT_LT):
                k_ps = pp.tile([P, 512], f32, tag="pp", name="k_ps")[:KT_LT, :256]
                for kt in range(8):
                    nc.tensor.matmul(
                        k_ps, xt[:, kt, lt * KT_LT:(lt + 1) * KT_LT], wkT[:, kt],
                        start=(kt == 0), stop=(kt == 7),
                    )
                raw = rp.tile([BQ, 512], bf16, tag="raw", name="raw")[:KT_LT, :256]
                nc.scalar.copy(raw[:], k_ps[:])
                raws.append((raw, KT_LT, N_KV_HEADS))
            # V projections for every block's key window (PE filler work
            # during the norm/transpose phases)
            vbs = []
            for b_ in range(NB):
                qs = b_ * BQ
                v_ps = pp.tile([P, 512], f32, tag="pp", name="v_ps")[:, :256]
                for kt in range(8):
                    nc.tensor.matmul(
                        v_ps, xt[:, kt, qs:qs + BK], wvT[:, kt],
                        start=(kt == 0), stop=(kt == 7),
                    )
                vb = vp.tile([P, 256], bf16, tag="vb", name="vb")
                nc.scalar.copy(vb[:], v_ps[:])
                vbs.append(vb)
            for b_ in range(NB):
                qs = b_ * BQ
                for ch in range(2):
                    q_ps = pp.tile([P, 512], f32, tag="pp", name="q_ps")[:BQ]
                    for kt in range(8):
                        nc.tensor.matmul(
                            q_ps,
                            xt[:, kt, HALO + qs:HALO + qs + BQ],
                            wqT[:, kt, ch * 512:(ch + 1) * 512],
                            start=(kt == 0), stop=(kt == 7),
                        )
                    raw = rp.tile([BQ, 512], bf16, tag="raw", name="q_raw")
                    nc.scalar.copy(raw[:], q_ps[:])
                    raws.append((raw, BQ, 8))

            # ---- phase 2: RMSNorm stats, batched per ACT function ----
            sqs, sss, rstds = [], [], []
            for raw, lpart, n_h in raws:
                sq = sp.tile([BQ, 512], bf16, tag="sq", name="sq")[:lpart, :n_h * 64]
                nc.scalar.square(sq[:], raw[:])
                sqs.append(sq)
            for (raw, lpart, n_h), sq in zip(raws, sqs):
                ss = mp.tile([BQ, 8], f32, tag="ss", name="ss")[:lpart, :n_h]
                nc.vector.reduce_sum(
                    ss[:], sq[:].rearrange("l (h d) -> l h d", d=HEAD_DIM),
                    axis=mybir.AxisListType.X,
                )
                nc.vector.tensor_scalar_add(ss[:], ss[:], HEAD_DIM * EPS)
                sss.append(ss)
            srts = []
            for (raw, lpart, n_h), ss in zip(raws, sss):
                srt = mp.tile([BQ, 8], f32, tag="srt", name="srt")[:lpart, :n_h]
                nc.scalar.activation(
                    srt[:], ss[:], mybir.ActivationFunctionType.Sqrt,
                    scale=1.0 / HEAD_DIM,
                )
                srts.append(srt)
            hats = []
            for (raw, lpart, n_h), srt in zip(raws, srts):
                rstd = mp.tile([BQ, 8], f32, tag="rstd", name="rstd")[:lpart, :n_h]
                nc.vector.reciprocal(rstd[:], srt[:])
                hat = rp.tile([BQ, 512], bf16, tag="hat", name="hat")[:lpart, :n_h * 64]
                nc.vector.tensor_tensor(
                    hat[:].rearrange("l (h d) -> l h d", d=HEAD_DIM),
                    raw[:].rearrange("l (h d) -> l h d", d=HEAD_DIM),
                    rstd[:, :, None].to_broadcast((lpart, n_h, HEAD_DIM)),
                    mybir.AluOpType.mult,
                )
                hats.append(hat)

            # ---- phase 3: PE transposes -> KT / QT ----
            idx = 0
            for lt in range(LX // KT_LT):
                hat = hats[idx]; idx += 1
                tp = pp.tile([P, 2, BQ], bf16, tag="pp", name="tp")[:, :, :KT_LT]
                for ot in range(2):
                    nc.tensor.transpose(
                        tp[:, ot], hat[:, ot * P:(ot + 1) * P],
                        ident[:KT_LT, :KT_LT])
                dst = KT[:, :, lt * KT_LT:(lt + 1) * KT_LT]
                if lt % 2 == 0:
                    nc.vector.tensor_copy(dst, tp[:])
                else:
                    nc.scalar.copy(dst, tp[:])
            for b_ in range(NB):
                qs = b_ * BQ
                for ch in range(2):
                    hat = hats[idx]; idx += 1
                    tp = pp.tile([P, 4, BQ], bf16, tag="pp", name="tpq")
                    for ot in range(4):
                        nc.tensor.transpose(
                            tp[:, ot], hat[:, ot * P:(ot + 1) * P], ident[:])
                    dst = QT[:, ch * 4:ch * 4 + 4, qs:qs + BQ]
                    if ch % 2 == 0:
                        nc.vector.tensor_copy(dst, tp[:])
                    else:
                        nc.scalar.copy(dst, tp[:])
            if apply_u:
                kts = gp.tile([P, 2, LX], bf16)
                for ot in range(2):
                    nc.scalar.activation(
                        kts[:, ot], KT[:, ot],
                        mybir.ActivationFunctionType.Copy, scale=uvec[:],
                    )
                KT = kts

            # ---- phase 4: attention + output projection per block ----
            for b_ in range(NB):
                qs = b_ * BQ
                ebT = ebT0 if b_ == 0 else ebTr
                vb = vbs[b_]

                ot_ps = pbig.tile([P, 8, P], f32, tag="big", name="ot_ps")
                rcps = []
                for half in range(2):
                    # scores, pre-transposed: S.T[jk, slot, i]
                    sc = pbig.tile([P, 8, P], f32, tag="big", name="sc")
                    for t in range(8):
                        h = 2 * t + half
                        g = h % N_KV_HEADS
                        nc.tensor.matmul(
                            sc[:, t, :BQ],
                            KT[(g % 2) * 64:(g % 2) * 64 + 64, g // 2, qs:qs + BK],
                            QT[(h % 2) * 64:(h % 2) * 64 + 64, h // 2, qs:qs + BQ],
                            start=True, stop=True,
                        )
                    e_t = ap.tile([P, 8, BQ], bf16, tag="et")
                    nc.scalar.activation(
                        e_t[:], sc[:, :, :BQ],
                        mybir.ActivationFunctionType.Exp, scale=0.125,
                    )
                    ptr = ap.tile([P, 8, BQ], bf16, tag="ptr")
                    nc.vector.tensor_tensor(
                        ptr[:], e_t[:], ebT[:, half * 8:half * 8 + 8, :],
                        mybir.AluOpType.mult,
                    )
                    # AV: out rows (h%2)*64..+64 of pair t
                    for t in range(8):
                        h = 2 * t + half
                        g = h % N_KV_HEADS
                        nc.tensor.matmul(
                            ot_ps[half * 64:half * 64 + 64, t, :BQ],
                            vb[:, g * 64:(g + 1) * 64],
                            ptr[:, t, :],
                            start=True, stop=True,
                        )
                    # denominators, replicated over 64 partitions by the
                    # ones-matmul; reciprocal lands in SBUF for the fused
                    # normalize-evict multiply
                    rcp = sp.tile([64, 8, BQ], f32, tag="rcp", name="rcp")
                    for c in range(2):
                        den = pp.tile([P, 512], f32, tag="pp", name="den")[:64, :4 * BQ]
                        nc.tensor.matmul(
                            den, ones64[:], ptr[:, 4 * c:4 * c + 4, :],
                            start=True, stop=True,
                        )
                        nc.vector.reciprocal_approx_fast(
                            rcp[:, 4 * c:4 * c + 4, :],
                            den[:].rearrange("p (s i) -> p s i", i=BQ),
                        )
                    rcps.append(rcp)
                for half in range(2):
                    nc.vector.tensor_tensor(
                        OT[half * 64:half * 64 + 64, :, qs:qs + BQ],
                        ot_ps[half * 64:half * 64 + 64, :, :BQ],
                        rcps[half][:],
                        mybir.AluOpType.mult,
                    )

                # output projection for this block
                nrows = BQ if b_ < NB - 1 else LQ - (NB - 1) * BQ
                for ch in range(2):
                    y_ps = pp.tile([P, 512], f32, tag="pp", name="y_ps")[:BQ]
                    for ot in range(8):
                        nc.tensor.matmul(
                            y_ps, OT[:, ot, qs:qs + BQ],
                            woT[:, ot, ch * 512:(ch + 1) * 512],
                            start=(ot == 0), stop=(ot == 7),
                        )
                    y_sb = sp.tile([BQ, 512], f32, tag="ysb", name="ysb")
                    nc.scalar.copy(y_sb[:], y_ps[:])
                    nc.sync.dma_start(
                        out_e.ap()[qs:qs + nrows, ch * 512:(ch + 1) * 512],
                        y_sb[:nrows],
                    )
    nc.compile()
    return nc


def _shard_inputs(x, wq, wk, wv, wo, q_norm_w, k_norm_w):
    u = (np.asarray(q_norm_w, np.float32) * np.asarray(k_norm_w, np.float32))
    apply_u = not np.allclose(u, 1.0)

    def ktile(wT):  # [DIM, O] -> [128, 8, O] bf16 (k-tiled)
        return np.ascontiguousarray(
            wT.astype(BF16).reshape(8, P, -1).transpose(1, 0, 2))

    wqT = ktile(np.asarray(wq, np.float32).T)
    wkT = ktile(np.asarray(wk, np.float32).T)
    wvT = ktile(np.asarray(wv, np.float32).T)
    woT = ktile(np.asarray(wo, np.float32).T)  # wo[e, o] -> [o, e], contraction o
    uvec = np.tile(u, 2).reshape(P, 1).astype(np.float32)
    ebTr = _exp_bias_t(edge=False)
    ident = np.eye(BQ, dtype=np.float32).astype(BF16)

    in_maps = []
    for c in range(8):
        b, j = c // 4, c % 4
        xh = np.zeros((LX, DIM), np.float32)
        lo = j * LQ - HALO
        s0, s1 = max(lo, 0), min(j * LQ + NB * BQ, L)
        xh[s0 - lo:s1 - lo] = x[b, s0:s1]
        xtc = np.ascontiguousarray(
            xh.T.astype(BF16).reshape(8, P, LX).transpose(1, 0, 2))
        ebT0 = _exp_bias_t(edge=(j == 0))
        in_maps.append({
            "xt": xtc, "wqT": wqT, "wkT": wkT, "wvT": wvT, "woT": woT,
            "ebT0": ebT0, "ebTr": ebTr, "ident": ident, "uvec": uvec,
        })
    return in_maps, apply_u


def _run(inputs, trace=False):
    global _compiled
    in_maps, apply_u = _shard_inputs(**inputs)
    if _compiled is None or _compiled[1] != apply_u:
        _compiled = (_build(apply_u), apply_u)
    nc = _compiled[0]
    res = run_bass_kernel_spmd(nc, in_maps, list(range(8)), trace=trace)
    full = np.empty((B, L, DIM), np.float32)
    for c in range(8):
        b, j = c // 4, c % 4
        full[b, j * LQ:(j + 1) * LQ] = res.results[c]["out"]
    return full, res


def kernel(x, wq, wk, wv, wo, q_norm_w, k_norm_w):
    full, _ = _run(dict(x=np.asarray(x), wq=np.asarray(wq), wk=np.asarray(wk),
                        wv=np.asarray(wv), wo=np.asarray(wo),
                        q_norm_w=np.asarray(q_norm_w),
                        k_norm_w=np.asarray(k_norm_w)))
    return full


# revision 10
# speedup vs baseline: 1.1836x; 1.1432x over previous
"""ALiBi sliding-window GQA attention on 8 Trainium2 NeuronCores.

Sharding: batch (2) x sequence quarter (4) -> 8 cores, each computing a
disjoint [512, 1024] output chunk from a 528-token input slice (16-token
halo on the left for the sliding window). No collectives needed.

Per-core kernel (bf16 compute, f32 accumulate), v2 dataflow:
  1. Q/K projections in [token, feature] layout, staged raw to SBUF via ACT
     copies; RMSNorm stats batched per activation function (no LUT thrash).
  2. Normalized Q/K transposed to [feature, token] via PE transpose-mode
     (no XBAR DMA transposes - those serialize ~1.2us each on SP/ACT).
  3. 5 query blocks of 112 with 128-key windows, heads in 2 half-phases
     (even heads then odd heads so packed PE row-groups never share a
     PSUM bank):
       scores come out PRE-TRANSPOSED: S.T[key, head-slot, query] =
         K^T.T @ Q^T, so softmax probabilities feed the AV matmul with no
         per-head transpose at all.
       P.T = exp(S.T/8) * ebT (host table: ALiBi slopes + causal/window
         mask as multiplicative zeros, pre-transposed, slot-permuted)
       row sums via ones-matmul (also replicates them across 64
         partitions); normalization by reciprocal-multiply fused into the
         AV PSUM->SBUF eviction.
  4. Output projection uses OUT^T as the stationary operand so results land
     in [token, feature] layout for contiguous stores.
"""

import math

import numpy as np
import ml_dtypes

import concourse.bass as bass
import concourse.tile as tile
from concourse import bacc, mybir
from concourse.bass_utils import run_bass_kernel_spmd

BF16 = ml_dtypes.bfloat16

B, L, DIM = 2, 2048, 1024
N_HEADS, N_KV_HEADS, HEAD_DIM = 16, 4, 64
WINDOW = 16
EPS = 0.01

LQ = 512           # queries per core
HALO = WINDOW      # left halo
NB = 5             # query blocks per core
BQ = 112           # queries per block
BK = 128           # key window per block
LX = HALO + NB * BQ  # 576 = padded x slice width per core
KT_LT = 96         # K projection l-tile
P = 128

# head h -> scores slot: even heads -> slots 0..7, odd -> 8..15 (pair order)
SLOT = [h // 2 + 8 * (h % 2) for h in range(N_HEADS)]

_compiled = None


def _alibi_slopes(n_heads):
    closest = 2 ** math.floor(math.log2(n_heads))
    base = 2.0 ** (-(2.0 ** (-(math.log2(closest) - 3))))
    slopes = base ** np.arange(1, closest + 1, dtype=np.float64)
    if closest < n_heads:
        eb = 2.0 ** (-(2.0 ** (-(math.log2(2 * closest) - 3))))
        extra = eb ** np.arange(1, 2 * (n_heads - closest) + 1, 2, dtype=np.float64)
        slopes = np.concatenate([slopes, extra])
    return slopes[:n_heads]


def _exp_bias_t(edge: bool) -> np.ndarray:
    """[BK, N_HEADS, BQ] transposed multiplicative softmax bias, slot order.

    Query i (block-local) sits at window column jk in [i, i+16]; entry is
    exp(slope_h * (jk - 16 - i)) inside the band, 0 outside.  With
    edge=True (first block of the sequence) keys at global position < 0
    (jk < 16) are additionally masked.
    """
    slopes = _alibi_slopes(N_HEADS)
    i = np.arange(BQ)[:, None]
    jk = np.arange(BK)[None, :]
    rel = jk - WINDOW - i                      # [BQ, BK]
    valid = (rel <= 0) & (rel >= -WINDOW)
    if edge:
        valid = valid & (jk >= WINDOW)
    arg = np.where(valid[None], slopes[:, None, None] * rel[None], -np.inf)
    eb = np.exp(arg)                           # [H, BQ, BK]
    perm = np.empty(N_HEADS, np.int64)
    perm[SLOT] = np.arange(N_HEADS)            # slot s holds head perm[s]
    return np.ascontiguousarray(eb[perm].transpose(2, 0, 1)).astype(BF16)


def _build(apply_u: bool):
    """Build the SPMD Bass program. apply_u folds q_norm_w*k_norm_w into K^T."""
    nc = bacc.Bacc("TRN2", target_bir_lowering=False, debug=False)
    f32, bf16 = mybir.dt.float32, mybir.dt.bfloat16

    xt_e = nc.dram_tensor("xt", [P, 8, LX], bf16, kind="ExternalInput")
    wq_e = nc.dram_tensor("wqT", [P, 8, 1024], bf16, kind="ExternalInput")
    wk_e = nc.dram_tensor("wkT", [P, 8, 256], bf16, kind="ExternalInput")
    wv_e = nc.dram_tensor("wvT", [P, 8, 256], bf16, kind="ExternalInput")
    wo_e = nc.dram_tensor("woT", [P, 8, 1024], bf16, kind="ExternalInput")
    eb0_e = nc.dram_tensor("ebT0", [BK, N_HEADS, BQ], bf16, kind="ExternalInput")
    ebr_e = nc.dram_tensor("ebTr", [BK, N_HEADS, BQ], bf16, kind="ExternalInput")
    id_e = nc.dram_tensor("ident", [BQ, BQ], bf16, kind="ExternalInput")
    u_e = nc.dram_tensor("uvec", [P, 1], f32, kind="ExternalInput")
    out_e = nc.dram_tensor("out", [LQ, DIM], f32, kind="ExternalOutput")

    NT = LX // KT_LT + 2 * NB  # 6 K-tiles + 10 Q-chunks

    with tile.TileContext(nc) as tc:
        with (
            tc.tile_pool(name="w", bufs=1) as wp,
            tc.tile_pool(name="glob", bufs=1) as gp,
            tc.tile_pool(name="raw", bufs=NT) as rp,
            tc.tile_pool(name="stage", bufs=4) as sp,
            tc.tile_pool(name="small", bufs=NT) as mp,
            tc.tile_pool(name="att", bufs=2) as ap,
            tc.tile_pool(name="vpool", bufs=NB) as vp,
            tc.tile_pool(name="pp", bufs=2, space="PSUM") as pp,
            tc.tile_pool(name="pbig", bufs=3, space="PSUM") as pbig,
        ):
            # ---- PE warmup: junk matmuls fill the DMA-bound prologue so
            # the HAM clock-gate opens before real work arrives ----
            junk = wp.tile([P, 512], bf16)
            nc.vector.memset(junk[:], 1.0)
            ones64 = wp.tile([P, 64], bf16)
            nc.vector.memset(ones64[:], 1.0)
            wps = pp.tile([P, 512], f32, tag="pp", name="warm")[:64]
            for _ in range(16):
                nc.tensor.matmul(wps, ones64[:], junk[:], start=True, stop=True)

            # ---- input loads (per k-tile for fine-grained deps) ----
            xt = wp.tile([P, 8, LX], bf16)
            wkT = wp.tile([P, 8, 256], bf16)
            wqT = wp.tile([P, 8, 1024], bf16)
            wvT = wp.tile([P, 8, 256], bf16)
            woT = wp.tile([P, 8, 1024], bf16)
            # coarse DMAs: one descriptor per partition (the per-kt variant
            # costs ~60ns/row-descriptor and is descriptor-bound)
            for g in range(2):
                nc.sync.dma_start(xt[:, 4 * g:4 * g + 4], xt_e.ap()[:, 4 * g:4 * g + 4])
            nc.sync.dma_start(wkT[:], wk_e.ap())
            nc.sync.dma_start(wvT[:], wv_e.ap())
            for g in range(4):
                nc.sync.dma_start(wqT[:, 2 * g:2 * g + 2], wq_e.ap()[:, 2 * g:2 * g + 2])
            ebT0 = wp.tile([BK, N_HEADS, BQ], bf16)
            ebTr = wp.tile([BK, N_HEADS, BQ], bf16)
            ident = wp.tile([BQ, BQ], bf16)
            nc.sync.dma_start(ebT0[:], eb0_e.ap())
            nc.sync.dma_start(ebTr[:], ebr_e.ap())
            nc.sync.dma_start(ident[:], id_e.ap())
            uvec = wp.tile([P, 1], f32)
            if apply_u:
                nc.sync.dma_start(uvec[:], u_e.ap())
            for g in range(4):
                nc.sync.dma_start(woT[:, 2 * g:2 * g + 2], wo_e.ap()[:, 2 * g:2 * g + 2])

            QT = gp.tile([P, 8, LX], bf16)    # normalized Q transposed
            KT = gp.tile([P, 2, LX], bf16)    # normalized K transposed
            OT = gp.tile([P, 8, LX], bf16)    # attention out transposed

            # ---- phase 1: K/Q projections -> raw SBUF (ACT Copy only) ----
            # tiles: 6 K l-tiles of 96, then 10 Q (block, half) chunks of
            # [112, 512]
            raws = []
            for lt in range(LX // KT_LT):
                k_ps = pp.tile([P, 512], f32, tag="pp", name="k_ps")[:KT_LT, :256]
                for kt in range(8):
                    nc.tensor.matmul(
                        k_ps, xt[:, kt, lt * KT_LT:(lt + 1) * KT_LT], wkT[:, kt],
                        start=(kt == 0), stop=(kt == 7),
                    )
                raw = rp.tile([BQ, 512], bf16, tag="raw", name="raw")[:KT_LT, :256]
                nc.scalar.copy(raw[:], k_ps[:])
                raws.append((raw, KT_LT, N_KV_HEADS))
            # V projections for every block's key window (PE filler work
            # during the norm/transpose phases)
            vbs = []
            for b_ in range(NB):
                qs = b_ * BQ
                v_ps = pp.tile([P, 512], f32, tag="pp", name="v_ps")[:, :256]
                for kt in range(8):
                    nc.tensor.matmul(
                        v_ps, xt[:, kt, qs:qs + BK], wvT[:, kt],
                        start=(kt == 0), stop=(kt == 7),
                    )
                vb = vp.tile([P, 256], bf16, tag="vb", name="vb")
                nc.scalar.copy(vb[:], v_ps[:])
                vbs.append(vb)
            for b_ in range(NB):
                qs = b_ * BQ
                for ch in range(2):
                    q_ps = pp.tile([P, 512], f32, tag="pp", name="q_ps")[:BQ]
                    for kt in range(8):
                        nc.tensor.matmul(
                            q_ps,
                            xt[:, kt, HALO + qs:HALO + qs + BQ],
                            wqT[:, kt, ch * 512:(ch + 1) * 512],
                            start=(kt == 0), stop=(kt == 7),
                        )
                    raw = rp.tile([BQ, 512], bf16, tag="raw", name="q_raw")
                    nc.scalar.copy(raw[:], q_ps[:])
                    raws.append((raw, BQ, 8))

            # ---- phase 2: RMSNorm stats, batched per ACT function ----
            sqs, sss, rstds = [], [], []
            for raw, lpart, n_h in raws:
                sq = sp.tile([BQ, 512], bf16, tag="sq", name="sq")[:lpart, :n_h * 64]
                nc.scalar.square(sq[:], raw[:])
                sqs.append(sq)
            for (raw, lpart, n_h), sq in zip(raws, sqs):
                ss = mp.tile([BQ, 8], f32, tag="ss", name="ss")[:lpart, :n_h]
                nc.vector.reduce_sum(
                    ss[:], sq[:].rearrange("l (h d) -> l h d", d=HEAD_DIM),
                    axis=mybir.AxisListType.X,
                )
                nc.vector.tensor_scalar_add(ss[:], ss[:], HEAD_DIM * EPS)
                sss.append(ss)
            srts = []
            for (raw, lpart, n_h), ss in zip(raws, sss):
                srt = mp.tile([BQ, 8], f32, tag="srt", name="srt")[:lpart, :n_h]
                nc.scalar.activation(
                    srt[:], ss[:], mybir.ActivationFunctionType.Sqrt,
                    scale=1.0 / HEAD_DIM,
                )
                srts.append(srt)
            hats = []
            for (raw, lpart, n_h), srt in zip(raws, srts):
                rstd = mp.tile([BQ, 8], f32, tag="rstd", name="rstd")[:lpart, :n_h]
                nc.vector.reciprocal(rstd[:], srt[:])
                hat = rp.tile([BQ, 512], bf16, tag="hat", name="hat")[:lpart, :n_h * 64]
                nc.vector.tensor_tensor(
                    hat[:].rearrange("l (h d) -> l h d", d=HEAD_DIM),
                    raw[:].rearrange("l (h d) -> l h d", d=HEAD_DIM),
                    rstd[:, :, None].to_broadcast((lpart, n_h, HEAD_DIM)),
                    mybir.AluOpType.mult,
                )
                hats.append(hat)

            # ---- phase 3: PE transposes -> KT / QT ----
            idx = 0
            for lt in range(LX // KT_LT):
                hat = hats[idx]; idx += 1
                tp = pp.tile([P, 2, BQ], bf16, tag="pp", name="tp")[:, :, :KT_LT]
                for ot in range(2):
                    nc.tensor.transpose(
                        tp[:, ot], hat[:, ot * P:(ot + 1) * P],
                        ident[:KT_LT, :KT_LT])
                dst = KT[:, :, lt * KT_LT:(lt + 1) * KT_LT]
                if lt % 2 == 0:
                    nc.vector.tensor_copy(dst, tp[:])
                else:
                    nc.scalar.copy(dst, tp[:])
            for b_ in range(NB):
                qs = b_ * BQ
                for ch in range(2):
                    hat = hats[idx]; idx += 1
                    tp = pp.tile([P, 4, BQ], bf16, tag="pp", name="tpq")
                    for ot in range(4):
                        nc.tensor.transpose(
                            tp[:, ot], hat[:, ot * P:(ot + 1) * P], ident[:])
                    dst = QT[:, ch * 4:ch * 4 + 4, qs:qs + BQ]
                    if ch % 2 == 0:
                        nc.vector.tensor_copy(dst, tp[:])
                    else:
                        nc.scalar.copy(dst, tp[:])
            if apply_u:
                kts = gp.tile([P, 2, LX], bf16)
                for ot in range(2):
                    nc.scalar.activation(
                        kts[:, ot], KT[:, ot],
                        mybir.ActivationFunctionType.Copy, scale=uvec[:],
                    )
                KT = kts

            # ---- phase 4: attention + output projection per block ----
            for b_ in range(NB):
                qs = b_ * BQ
                ebT = ebT0 if b_ == 0 else ebTr
                vb = vbs[b_]

                ot_ps = pbig.tile([P, 8, P], f32, tag="big", name="ot_ps")
                rcps = []
                for half in range(2):
                    # scores, pre-transposed: S.T[jk, slot, i]
                    sc = pbig.tile([P, 8, P], f32, tag="big", name="sc")
                    for t in range(8):
                        h = 2 * t + half
                        g = h % N_KV_HEADS
                        nc.tensor.matmul(
                            sc[:, t, :BQ],
                            KT[(g % 2) * 64:(g % 2) * 64 + 64, g // 2, qs:qs + BK],
                            QT[(h % 2) * 64:(h % 2) * 64 + 64, h // 2, qs:qs + BQ],
                            start=True, stop=True,
                        )
                    e_t = ap.tile([P, 8, BQ], bf16, tag="et")
                    nc.scalar.activation(
                        e_t[:], sc[:, :, :BQ],
                        mybir.ActivationFunctionType.Exp, scale=0.125,
                    )
                    ptr = ap.tile([P, 8, BQ], bf16, tag="ptr")
                    nc.vector.tensor_tensor(
                        ptr[:], e_t[:], ebT[:, half * 8:half * 8 + 8, :],
                        mybir.AluOpType.mult,
                    )
                    # AV: out rows (h%2)*64..+64 of pair t
                    for t in range(8):
                        h = 2 * t + half
                        g = h % N_KV_HEADS
                        nc.tensor.matmul(
                            ot_ps[half * 64:half * 64 + 64, t, :BQ],
                            vb[:, g * 64:(g + 1) * 64],
                            ptr[:, t, :],
                            start=True, stop=True,
                        )
                    # denominators, replicated over 64 partitions by the
                    # ones-matmul; reciprocal lands in SBUF for the fused
                    # normalize-evict multiply
                    rcp = sp.tile([64, 8, BQ], f32, tag="rcp", name="rcp")
                    for c in range(2):
                        den = pp.tile([P, 512], f32, tag="pp", name="den")[:64, :4 * BQ]
                        nc.tensor.matmul(
                            den, ones64[:], ptr[:, 4 * c:4 * c + 4, :],
                            start=True, stop=True,
                        )
                        nc.vector.reciprocal_approx_fast(
                            rcp[:, 4 * c:4 * c + 4, :],
                            den[:].rearrange("p (s i) -> p s i", i=BQ),
                        )
                    rcps.append(rcp)
                for half in range(2):
                    nc.vector.tensor_tensor(
                        OT[half * 64:half * 64 + 64, :, qs:qs + BQ],
                        ot_ps[half * 64:half * 64 + 64, :, :BQ],
                        rcps[half][:],
                        mybir.AluOpType.mult,
                    )

                # output projection for this block
                nrows = BQ if b_ < NB - 1 else LQ - (NB - 1) * BQ
                for ch in range(2):
                    y_ps = pp.tile([P, 512], f32, tag="pp", name="y_ps")[:BQ]
                    for ot in range(8):
                        nc.tensor.matmul(
                            y_ps, OT[:, ot, qs:qs + BQ],
                            woT[:, ot, ch * 512:(ch + 1) * 512],
                            start=(ot == 0), stop=(ot == 7),
                        )
                    y_sb = sp.tile([BQ, 512], f32, tag="ysb", name="ysb")
                    nc.scalar.copy(y_sb[:], y_ps[:])
                    nc.sync.dma_start(
                        out_e.ap()[qs:qs + nrows, ch * 512:(ch + 1) * 512],
                        y_sb[:nrows],
                    )
    nc.compile()
    return nc


def _shard_inputs(x, wq, wk, wv, wo, q_norm_w, k_norm_w):
    u = (np.asarray(q_norm_w, np.float32) * np.asarray(k_norm_w, np.float32))
    apply_u = not np.allclose(u, 1.0)

    def ktile(wT):  # [DIM, O] -> [128, 8, O] bf16 (k-tiled)
        return np.ascontiguousarray(
            wT.astype(BF16).reshape(8, P, -1).transpose(1, 0, 2))

    wqT = ktile(np.asarray(wq, np.float32).T)
    wkT = ktile(np.asarray(wk, np.float32).T)
    wvT = ktile(np.asarray(wv, np.float32).T)
    woT = ktile(np.asarray(wo, np.float32).T)  # wo[e, o] -> [o, e], contraction o
    uvec = np.tile(u, 2).reshape(P, 1).astype(np.float32)
    ebTr = _exp_bias_t(edge=False)
    ident = np.eye(BQ, dtype=np.float32).astype(BF16)

    in_maps = []
    for c in range(8):
        b, j = c // 4, c % 4
        xh = np.zeros((LX, DIM), np.float32)
        lo = j * LQ - HALO
        s0, s1 = max(lo, 0), min(j * LQ + NB * BQ, L)
        xh[s0 - lo:s1 - lo] = x[b, s0:s1]
        xtc = np.ascontiguousarray(
            xh.T.astype(BF16).reshape(8, P, LX).transpose(1, 0, 2))
        ebT0 = _exp_bias_t(edge=(j == 0))
        in_maps.append({
            "xt": xtc, "wqT": wqT, "wkT": wkT, "wvT": wvT, "woT": woT,
            "ebT0": ebT0, "ebTr": ebTr, "ident": ident, "uvec": uvec,
        })
    return in_maps, apply_u


def _run(inputs, trace=False):
    global _compiled
    in_maps, apply_u = _shard_inputs(**inputs)
    if _compiled is None or _compiled[1] != apply_u:
        _compiled = (_build(apply_u), apply_u)
    nc = _compiled[0]
    res = run_bass_kernel_spmd(nc, in_maps, list(range(8)), trace=trace)
    full = np.empty((B, L, DIM), np.float32)
    for c in range(8):
        b, j = c // 4, c % 4
        full[b, j * LQ:(j + 1) * LQ] = res.results[c]["out"]
    return full, res


def kernel(x, wq, wk, wv, wo, q_norm_w, k_norm_w):
    full, _ = _run(dict(x=np.asarray(x), wq=np.asarray(wq), wk=np.asarray(wk),
                        wv=np.asarray(wv), wo=np.asarray(wo),
                        q_norm_w=np.asarray(q_norm_w),
                        k_norm_w=np.asarray(k_norm_w)))
    return full
